# revision 2
# baseline (speedup 1.0000x reference)
"""LoRA linear y = x @ (B@A).T computed low-rank: y = (x @ A.T) @ B.T.

Sharding: data-parallel over tokens (B*S = 16384) across 8 NeuronCores,
2048 tokens/core; lora_A / lora_B replicated (tiny). No collectives.

v2: bf16 end-to-end (rel-err gate is 2e-2; bf16 path lands ~2e-3).
Host casts x to bf16 AND pre-transposes it per core to xT [4096, 2048]
(packed [16, 128, 4096]: two 128-row d-chunks per 1MB DMA tile), so the
kernel needs zero on-chip transposes. HBM traffic/core: 16 MB in +
16 MB out = 32 MB vs 64 MB for the fp32 kernel -> DMA roofline ~89us.

Per-core pipeline:
  phase 1 (mm1): for each of 16 slab-pairs [128, 4096] (d-chunks 2g,2g+1
    x 2048 tokens), accumulate tT[16, 2048] in one 4-bank PSUM tile over
    all 32 d-chunks (lhsT = A.T chunk [128,16], rhs = xT slab cols).
  tT -> SBUF bf16 (DVE).
  phase 2 (mm2): for each token block h (16 x 128): 8 matmuls
    y[128,512] = tT_h.T @ BT_nb (K=16); PSUM->SBUF bf16 copies split
    DVE/ACT; store [128, 4096] (1 MB) on the scalar HWDGE ring.
Host casts the bf16 output back to fp32.
"""

import os
import numpy as np
import ml_dtypes

import concourse.bass as bass
import concourse.mybir as mybir
from concourse.tile import TileContext
from concourse.bass_utils import run_bass_kernel_spmd

N_CORES = 8
B, S, D_IN, D_OUT, R = 4, 4096, 4096, 4096, 16
TOK = B * S
TPC = TOK // N_CORES   # tokens per core: 2048
NC_D = D_IN // 128     # 32 d-chunks
NG = NC_D // 2         # 16 slab pair-groups
NH = TPC // 128        # 16 token blocks
BF16 = mybir.dt.bfloat16
F32 = mybir.dt.float32
NP_BF16 = ml_dtypes.bfloat16

XB = int(os.environ.get("XB", "3"))    # x slab bufs
YB = int(os.environ.get("YB", "3"))    # y tile bufs
YPB = int(os.environ.get("YPB", "4"))  # y PSUM bufs


def _split_drain_waits(nc):
    """This walrus build rejects instructions carrying >1 sem wait; hoist
    extra waits onto preceding single-wait NoOps on the same engine."""
    f = nc.m.functions[0]

    def fix_bb(bb):
        insts = getattr(bb, "instructions", None)
        if insts:
            new = []
            for inst in insts:
                si = inst.sync_info
                if si is not None and si.on_wait is not None and len(si.on_wait) > 1:
                    waits = list(si.on_wait)
                    for w in waits[:-1]:
                        d = mybir.InstNoOp(
                            name=nc.get_next_instruction_name(), ins=[], outs=[]
                        )
                        d.engine = inst.engine
                        d.sync_info = mybir.SyncInfo(on_wait=[w], on_update=[])
                        new.append(d)
                    si.on_wait = [waits[-1]]
                    inst.sync_info = si
                new.append(inst)
            bb.instructions[:] = new
        for sub in getattr(bb, "blocks", []) or []:
            fix_bb(sub)

    for blk in f.blocks:
        fix_bb(blk)


def _build():
    nc = bass.Bass("TRN2", target_bir_lowering=False, debug=False, num_devices=N_CORES)
    # xs[g, p, j*2048 + t] = x[token t, (2g+j)*128 + p]  (bf16)
    xs = nc.declare_dram_parameter("xs", [NG, 128, 2 * TPC], BF16, isOutput=False)
    # atp[p, c*R + r] = A[r, c*128 + p]  (bf16)
    atp = nc.declare_dram_parameter("atp", [128, NC_D * R], BF16, isOutput=False)
    bt = nc.declare_dram_parameter("bt", [R, D_OUT], BF16, isOutput=False)
    ys = nc.declare_dram_parameter("ys", [TPC, D_OUT], BF16, isOutput=True)

    with TileContext(nc) as tc:
        with (
            tc.tile_pool(name="const", bufs=1) as cpool,
            tc.tile_pool(name="x", bufs=XB) as xpool,
            tc.tile_pool(name="y", bufs=YB) as ypool,
            tc.tile_pool(name="t", bufs=1) as tpool,
            tc.tile_pool(name="t_ps", bufs=1, space="PSUM") as tpsum,
            tc.tile_pool(name="y_ps", bufs=YPB, space="PSUM") as ypsum,
        ):
            at_sb = cpool.tile([128, NC_D * R], BF16)
            nc.scalar.dma_start(out=at_sb[:], in_=atp[:])
            bt_sb = cpool.tile([R, D_OUT], BF16)
            nc.scalar.dma_start(out=bt_sb[:], in_=bt[:])

            # ---- phase 1: tT[16, 2048] = A @ x_shard.T, PSUM-accumulated
            tps = tpsum.tile([R, TPC], F32)  # 4 PSUM banks
            for g in range(NG):
                slab = xpool.tile([128, 2 * TPC], BF16)
                nc.sync.dma_start(out=slab[:], in_=xs[g])
                for j in range(2):
                    c = 2 * g + j
                    for q in range(TPC // 512):
                        nc.tensor.matmul(
                            tps[:, q * 512 : (q + 1) * 512],
                            at_sb[:, c * R : (c + 1) * R],
                            slab[:, j * TPC + q * 512 : j * TPC + (q + 1) * 512],
                            start=(c == 0),
                            stop=(c == NC_D - 1),
                        )
            t_sb = tpool.tile([R, TPC], BF16)
            nc.vector.tensor_copy(out=t_sb[:], in_=tps[:])

            # ---- phase 2: y[2048, 4096] = t @ B.T, streamed by 128-token blocks
            for h in range(NH):
                y_sb = ypool.tile([128, D_OUT], BF16)
                for nb in range(D_OUT // 512):
                    yps = ypsum.tile([128, 512], F32)
                    nc.tensor.matmul(
                        yps[:],
                        t_sb[:, h * 128 : (h + 1) * 128],
                        bt_sb[:, nb * 512 : (nb + 1) * 512],
                        start=True,
                        stop=True,
                    )
                    if nb % 2 == 0:
                        nc.vector.tensor_copy(
                            out=y_sb[:, nb * 512 : (nb + 1) * 512], in_=yps[:]
                        )
                    else:
                        nc.scalar.activation(
                            out=y_sb[:, nb * 512 : (nb + 1) * 512],
                            in_=yps[:],
                            func=mybir.ActivationFunctionType.Identity,
                        )
                nc.scalar.dma_start(
                    out=ys[h * 128 : (h + 1) * 128, :], in_=y_sb[:]
                )

    _split_drain_waits(nc)
    return nc


_NC = None


def _get_nc():
    global _NC
    if _NC is None:
        _NC = _build()
    return _NC


def _prep_inputs(x, lora_A, lora_B):
    x2d = np.asarray(x, dtype=np.float32).reshape(TOK, D_IN)
    A = np.asarray(lora_A, dtype=np.float32)
    Bm = np.asarray(lora_B, dtype=np.float32)
    # atp[p, c*R + r] = A[r, c*128 + p]
    atp = np.ascontiguousarray(
        A.T.reshape(NC_D, 128, R).transpose(1, 0, 2).reshape(128, NC_D * R)
    ).astype(NP_BF16)
    btv = np.ascontiguousarray(Bm.T).astype(NP_BF16)
    xs_list = []
    for i in range(N_CORES):
        shard = x2d[i * TPC : (i + 1) * TPC].astype(NP_BF16)  # [2048, 4096]
        # -> [g, p, j, t] = shard[t, (2g+j)*128 + p]
        v = shard.reshape(TPC, NG, 2, 128).transpose(1, 3, 2, 0)
        xs_list.append(np.ascontiguousarray(v).reshape(NG, 128, 2 * TPC))
    return xs_list, atp, btv


def kernel(x, lora_A, lora_B, _trace=False, _trace_kwargs=None):
    nc = _get_nc()
    xs_list, atp, btv = _prep_inputs(x, lora_A, lora_B)
    in_maps = [
        {"xs": xs_list[i], "atp": atp, "bt": btv} for i in range(N_CORES)
    ]
    res = run_bass_kernel_spmd(
        nc, in_maps, list(range(N_CORES)), trace=_trace, **(_trace_kwargs or {})
    )
    y = np.concatenate(
        [np.asarray(res.results[i]["ys"]) for i in range(N_CORES)], axis=0
    )
    out = y.astype(np.float32).reshape(B, S, D_OUT)
    if _trace:
        return out, res
    return out


# revision 5
# speedup vs baseline: 1.2555x; 1.2555x over previous
"""LoRA linear y = x @ (B@A).T computed low-rank: y = (x @ A.T) @ B.T.

Sharding: data-parallel over tokens (B*S = 16384) across 8 NeuronCores,
2048 tokens/core; lora_A / lora_B replicated (tiny). No collectives.

bf16 end-to-end (rel-err gate 2e-2; this path lands ~3.5e-3). Host casts
x to bf16 and pre-transposes per core to xT (d-major), so the kernel
needs zero on-chip transposes. HBM traffic/core: 16 MB in + 16 MB out
-> DMA roofline ~89us.

v3 over v2 (161us):
 - rank dim zero-padded 16->128 for mm2 (t_pad rows 16-127 = 0, bt_pad
   rows 16-127 = 0): K=128 keeps the PE HAM clock-gate at 8/8 (2.4 GHz).
   v2's K=16 mm2 ran the whole store phase at 1.2 GHz.
 - token-split software pipeline (NSPLIT=2): mm1(s) slab loads+matmuls
   interleaved into mm2(s-1)'s h-loop, so stores(s-1) overlap loads(s)
   and HBM never idles between phases.
 - PSUM drains paired into [128,1024] copies alternating DVE/ACT;
   stores alternate sync/scalar HWDGE rings; deeper slab buffering.
"""

import os
import numpy as np
import ml_dtypes

import concourse.bass as bass
import concourse.mybir as mybir
from concourse.tile import TileContext
from concourse.bass_utils import run_bass_kernel_spmd

N_CORES = 8
B, S, D_IN, D_OUT, R = 4, 4096, 4096, 4096, 16
TOK = B * S
TPC = TOK // N_CORES        # tokens per core: 2048
NC_D = D_IN // 128          # 32 d-chunks
NG = NC_D // 2              # 16 slab pair-groups per split
BF16 = mybir.dt.bfloat16
F32 = mybir.dt.float32
NP_BF16 = ml_dtypes.bfloat16

NSPLIT = int(os.environ.get("NSPLIT", "2"))
TPS = TPC // NSPLIT         # tokens per split
NH_S = TPS // 128           # token blocks per split
NQ = TPS // 512             # 512-col matmul slices per split
XB = int(os.environ.get("XB", "6" if NSPLIT == 1 else "8"))
YB = int(os.environ.get("YB", "3"))


def _split_drain_waits(nc):
    """This walrus build rejects instructions carrying >1 sem wait; hoist
    extra waits onto preceding single-wait NoOps on the same engine."""
    f = nc.m.functions[0]

    def fix_bb(bb):
        insts = getattr(bb, "instructions", None)
        if insts:
            new = []
            for inst in insts:
                si = inst.sync_info
                if si is not None and si.on_wait is not None and len(si.on_wait) > 1:
                    waits = list(si.on_wait)
                    for w in waits[:-1]:
                        d = mybir.InstNoOp(
                            name=nc.get_next_instruction_name(), ins=[], outs=[]
                        )
                        d.engine = inst.engine
                        d.sync_info = mybir.SyncInfo(on_wait=[w], on_update=[])
                        new.append(d)
                    si.on_wait = [waits[-1]]
                    inst.sync_info = si
                new.append(inst)
            bb.instructions[:] = new
        for sub in getattr(bb, "blocks", []) or []:
            fix_bb(sub)

    for blk in f.blocks:
        fix_bb(blk)


def _build():
    nc = bass.Bass("TRN2", target_bir_lowering=False, debug=False, num_devices=N_CORES)
    # xs[s, g, p, j*TPS + t] = x[token s*TPS+t, (2g+j)*128 + p]  (bf16)
    xs = nc.declare_dram_parameter("xs", [NSPLIT, NG, 128, 2 * TPS], BF16, isOutput=False)
    # atp[p, c*R + r] = A[r, c*128 + p]  (bf16)
    atp = nc.declare_dram_parameter("atp", [128, NC_D * R], BF16, isOutput=False)
    bt = nc.declare_dram_parameter("bt", [R, D_OUT], BF16, isOutput=False)
    ys = nc.declare_dram_parameter("ys", [TPC, D_OUT], BF16, isOutput=True)

    with TileContext(nc) as tc:
        with (
            tc.tile_pool(name="const", bufs=1) as cpool,
            tc.tile_pool(name="x", bufs=XB) as xpool,
            tc.tile_pool(name="y", bufs=YB) as ypool,
            tc.tile_pool(name="t_ps", bufs=min(NSPLIT, 2), space="PSUM") as tpsum,
            tc.tile_pool(name="y_ps", bufs=2, space="PSUM") as ypsum,
        ):
            at_sb = cpool.tile([128, NC_D * R], BF16)
            nc.scalar.dma_start(out=at_sb[:], in_=atp[:])
            # bt_pad rows 16-127 zeroed on gpsimd (idle engine), rows 0-15 DMA'd
            bt_sb = cpool.tile([128, D_OUT], BF16)
            nc.gpsimd.memset(bt_sb[:], 0.0)
            nc.scalar.dma_start(out=bt_sb[0:R, :], in_=bt[:])
            # t_pad rows 16-127 stay zero for the whole kernel
            t_pad = cpool.tile([128, TPC], BF16)
            nc.vector.memset(t_pad[:], 0.0)

            tps_tiles = [None] * NSPLIT

            def emit_mm1_slab(s, g):
                if g == 0:
                    tps_tiles[s] = tpsum.tile([R, TPS], F32, name=f"tps{s}", tag="tps")
                tps = tps_tiles[s]
                slab = xpool.tile([128, 2 * TPS], BF16)
                nc.sync.dma_start(out=slab[:], in_=xs[s, g])
                for j in range(2):
                    c = 2 * g + j
                    for q in range(NQ):
                        nc.tensor.matmul(
                            tps[:, q * 512 : (q + 1) * 512],
                            at_sb[:, c * R : (c + 1) * R],
                            slab[:, j * TPS + q * 512 : j * TPS + (q + 1) * 512],
                            start=(c == 0),
                            stop=(c == NC_D - 1),
                        )

            def emit_tcopy(s):
                tps = tps_tiles[s]
                half = TPS // 2
                nc.vector.tensor_copy(
                    out=t_pad[0:R, s * TPS : s * TPS + half], in_=tps[:, 0:half]
                )
                nc.scalar.activation(
                    out=t_pad[0:R, s * TPS + half : (s + 1) * TPS],
                    in_=tps[:, half:TPS],
                    func=mybir.ActivationFunctionType.Identity,
                )

            def emit_mm2_h(s, h):
                row = s * TPS + h * 128
                y_sb = ypool.tile([128, D_OUT], BF16)
                for p in range(4):  # pairs of 512-wide matmuls
                    yps = ypsum.tile([128, 1024], F32)
                    for half in range(2):
                        nb = 2 * p + half
                        nc.tensor.matmul(
                            yps[:, half * 512 : (half + 1) * 512],
                            t_pad[:, row : row + 128],
                            bt_sb[:, nb * 512 : (nb + 1) * 512],
                            start=True,
                            stop=True,
                        )
                    if p % 2 == 0:
                        nc.vector.tensor_copy(
                            out=y_sb[:, p * 1024 : (p + 1) * 1024], in_=yps[:]
                        )
                    else:
                        nc.scalar.activation(
                            out=y_sb[:, p * 1024 : (p + 1) * 1024],
                            in_=yps[:],
                            func=mybir.ActivationFunctionType.Identity,
                        )
                eng = nc.sync if h % 2 == 0 else nc.scalar
                eng.dma_start(out=ys[row : row + 128, :], in_=y_sb[:])

            # software pipeline: mm1(s) interleaved into mm2(s-1)'s h-loop
            for g in range(NG):
                emit_mm1_slab(0, g)
            emit_tcopy(0)
            for s in range(1, NSPLIT):
                gg = 0
                for h in range(NH_S):
                    emit_mm2_h(s - 1, h)
                    for _ in range(NG // NH_S):
                        emit_mm1_slab(s, gg)
                        gg += 1
                emit_tcopy(s)
            for h in range(NH_S):
                emit_mm2_h(NSPLIT - 1, h)

    _split_drain_waits(nc)
    return nc


_NC = None


def _get_nc():
    global _NC
    if _NC is None:
        _NC = _build()
    return _NC


def _prep_inputs(x, lora_A, lora_B):
    x2d = np.asarray(x, dtype=np.float32).reshape(TOK, D_IN)
    A = np.asarray(lora_A, dtype=np.float32)
    Bm = np.asarray(lora_B, dtype=np.float32)
    atp = np.ascontiguousarray(
        A.T.reshape(NC_D, 128, R).transpose(1, 0, 2).reshape(128, NC_D * R)
    ).astype(NP_BF16)
    btv = np.ascontiguousarray(Bm.T).astype(NP_BF16)
    xs_list = []
    for i in range(N_CORES):
        shard = x2d[i * TPC : (i + 1) * TPC].astype(NP_BF16)  # [2048, 4096]
        # [s, t, g, j, p] -> [s, g, p, j, t]
        v = shard.reshape(NSPLIT, TPS, NG, 2, 128).transpose(0, 2, 4, 3, 1)
        xs_list.append(np.ascontiguousarray(v).reshape(NSPLIT, NG, 128, 2 * TPS))
    return xs_list, atp, btv


def kernel(x, lora_A, lora_B, _trace=False, _trace_kwargs=None):
    nc = _get_nc()
    xs_list, atp, btv = _prep_inputs(x, lora_A, lora_B)
    in_maps = [
        {"xs": xs_list[i], "atp": atp, "bt": btv} for i in range(N_CORES)
    ]
    res = run_bass_kernel_spmd(
        nc, in_maps, list(range(N_CORES)), trace=_trace, **(_trace_kwargs or {})
    )
    y = np.concatenate(
        [np.asarray(res.results[i]["ys"]) for i in range(N_CORES)], axis=0
    )
    out = y.astype(np.float32).reshape(B, S, D_OUT)
    if _trace:
        return out, res
    return out


# revision 6
# speedup vs baseline: 1.4005x; 1.1155x over previous
"""LoRA linear y = x @ (B@A).T computed low-rank: y = (x @ A.T) @ B.T.

Sharding: data-parallel over tokens (B*S = 16384) across 8 NeuronCores,
2048 tokens/core; lora_A / lora_B replicated (tiny). No collectives.

bf16 end-to-end (rel-err gate 2e-2; this path lands ~3.5e-3). Host casts
x to bf16 and pre-transposes per core to xT (d-major), so the kernel
needs zero on-chip transposes. HBM traffic/core: 16 MB in + 16 MB out
-> DMA roofline ~89us.

Pipeline (NSPLIT token splits/core, default 4 x 512 tokens):
 - mm1(s): tT_s[16, TPS] += A.T-chunk.T @ xT-slab over 32 d-chunks,
   slabs of 4 chunks ([128, 4*TPS] per DMA) on the sync ring.
 - tT -> rows 0-15 of t_pad (bf16, rows 16-127 zeroed once): mm2 runs
   with K zero-padded 16->128 so the PE HAM clock-gate sees full-row
   activity and stays at 8/8 (2.4 GHz). K=16 matmuls throttle to 1.2.
 - mm2(s): y[128,512] = t_pad_h.T @ bt_pad_nb into single-bank PSUM
   tiles (6-deep rotation; 2-bank tiles serialize back-to-back MMs),
   drains alternate DVE/ACT, stores alternate sync/scalar rings.
 - mm1(s+1) slab loads+matmuls are interleaved into mm2(s)'s h-loop so
   stores(s) overlap loads(s+1) and the PE never idles long enough to
   cool down between windows.
"""

import os
import numpy as np
import ml_dtypes

import concourse.bass as bass
import concourse.mybir as mybir
from concourse.tile import TileContext
from concourse.bass_utils import run_bass_kernel_spmd

N_CORES = 8
B, S, D_IN, D_OUT, R = 4, 4096, 4096, 4096, 16
TOK = B * S
TPC = TOK // N_CORES        # tokens per core: 2048
NC_D = D_IN // 128          # 32 d-chunks
CPS = 4                     # d-chunks per slab
NGS = NC_D // CPS           # 8 slabs per split
BF16 = mybir.dt.bfloat16
F32 = mybir.dt.float32
NP_BF16 = ml_dtypes.bfloat16

NSPLIT = int(os.environ.get("NSPLIT", "4"))
TPS = TPC // NSPLIT         # tokens per split
NH_S = TPS // 128           # token blocks per split
NQ = TPS // 512             # 512-col matmul slices per split
TPS_BANKS = (TPS * 4 + 2047) // 2048
XB = int(os.environ.get("XB", "10"))
YB = int(os.environ.get("YB", "3"))
YPB = int(os.environ.get("YPB", str(8 - 2 * TPS_BANKS)))


def _split_drain_waits(nc):
    """This walrus build rejects instructions carrying >1 sem wait; hoist
    extra waits onto preceding single-wait NoOps on the same engine."""
    f = nc.m.functions[0]

    def fix_bb(bb):
        insts = getattr(bb, "instructions", None)
        if insts:
            new = []
            for inst in insts:
                si = inst.sync_info
                if si is not None and si.on_wait is not None and len(si.on_wait) > 1:
                    waits = list(si.on_wait)
                    for w in waits[:-1]:
                        d = mybir.InstNoOp(
                            name=nc.get_next_instruction_name(), ins=[], outs=[]
                        )
                        d.engine = inst.engine
                        d.sync_info = mybir.SyncInfo(on_wait=[w], on_update=[])
                        new.append(d)
                    si.on_wait = [waits[-1]]
                    inst.sync_info = si
                new.append(inst)
            bb.instructions[:] = new
        for sub in getattr(bb, "blocks", []) or []:
            fix_bb(sub)

    for blk in f.blocks:
        fix_bb(blk)


def _build():
    nc = bass.Bass("TRN2", target_bir_lowering=False, debug=False, num_devices=N_CORES)
    # xs[s, g, p, j*TPS + t] = x[token s*TPS+t, (4g+j)*128 + p]  (bf16)
    xs = nc.declare_dram_parameter("xs", [NSPLIT, NGS, 128, CPS * TPS], BF16, isOutput=False)
    # atp[p, c*R + r] = A[r, c*128 + p]  (bf16)
    atp = nc.declare_dram_parameter("atp", [128, NC_D * R], BF16, isOutput=False)
    bt = nc.declare_dram_parameter("bt", [R, D_OUT], BF16, isOutput=False)
    ys = nc.declare_dram_parameter("ys", [TPC, D_OUT], BF16, isOutput=True)

    with TileContext(nc) as tc:
        with (
            tc.tile_pool(name="const", bufs=1) as cpool,
            tc.tile_pool(name="x", bufs=XB) as xpool,
            tc.tile_pool(name="y", bufs=YB) as ypool,
            tc.tile_pool(name="t_ps", bufs=min(NSPLIT, 2), space="PSUM") as tpsum,
            tc.tile_pool(name="y_ps", bufs=YPB, space="PSUM") as ypsum,
        ):
            at_sb = cpool.tile([128, NC_D * R], BF16)
            nc.scalar.dma_start(out=at_sb[:], in_=atp[:])
            # bt_pad rows 16-127 zeroed on gpsimd (idle engine), rows 0-15 DMA'd
            bt_sb = cpool.tile([128, D_OUT], BF16)
            nc.gpsimd.memset(bt_sb[:], 0.0)
            nc.scalar.dma_start(out=bt_sb[0:R, :], in_=bt[:])
            # t_pad rows 16-127 stay zero for the whole kernel
            t_pad = cpool.tile([128, TPC], BF16)
            nc.vector.memset(t_pad[:], 0.0)

            tps_tiles = [None] * NSPLIT

            def emit_mm1_slab(s, g):
                if g == 0:
                    tps_tiles[s] = tpsum.tile([R, TPS], F32, name=f"tps{s}", tag="tps")
                tps = tps_tiles[s]
                slab = xpool.tile([128, CPS * TPS], BF16)
                nc.sync.dma_start(out=slab[:], in_=xs[s, g])
                for j in range(CPS):
                    c = CPS * g + j
                    for q in range(NQ):
                        nc.tensor.matmul(
                            tps[:, q * 512 : (q + 1) * 512],
                            at_sb[:, c * R : (c + 1) * R],
                            slab[:, j * TPS + q * 512 : j * TPS + (q + 1) * 512],
                            start=(c == 0),
                            stop=(c == NC_D - 1),
                        )

            def emit_tcopy(s):
                tps = tps_tiles[s]
                half = TPS // 2
                nc.vector.tensor_copy(
                    out=t_pad[0:R, s * TPS : s * TPS + half], in_=tps[:, 0:half]
                )
                nc.scalar.activation(
                    out=t_pad[0:R, s * TPS + half : (s + 1) * TPS],
                    in_=tps[:, half:TPS],
                    func=mybir.ActivationFunctionType.Identity,
                )

            def emit_mm2_h(s, h):
                row = s * TPS + h * 128
                y_sb = ypool.tile([128, D_OUT], BF16)
                for nb in range(D_OUT // 512):
                    yps = ypsum.tile([128, 512], F32)
                    nc.tensor.matmul(
                        yps[:],
                        t_pad[:, row : row + 128],
                        bt_sb[:, nb * 512 : (nb + 1) * 512],
                        start=True,
                        stop=True,
                    )
                    if nb % 2 == 0:
                        nc.vector.tensor_copy(
                            out=y_sb[:, nb * 512 : (nb + 1) * 512], in_=yps[:]
                        )
                    else:
                        nc.scalar.activation(
                            out=y_sb[:, nb * 512 : (nb + 1) * 512],
                            in_=yps[:],
                            func=mybir.ActivationFunctionType.Identity,
                        )
                eng = nc.sync if h % 2 == 0 else nc.scalar
                eng.dma_start(out=ys[row : row + 128, :], in_=y_sb[:])

            # software pipeline: mm1(s) interleaved into mm2(s-1)'s h-loop
            for g in range(NGS):
                emit_mm1_slab(0, g)
            emit_tcopy(0)
            for s in range(1, NSPLIT):
                gg = 0
                for h in range(NH_S):
                    emit_mm2_h(s - 1, h)
                    for _ in range(NGS // NH_S):
                        emit_mm1_slab(s, gg)
                        gg += 1
                emit_tcopy(s)
            for h in range(NH_S):
                emit_mm2_h(NSPLIT - 1, h)

    _split_drain_waits(nc)
    return nc


_NC = None


def _get_nc():
    global _NC
    if _NC is None:
        _NC = _build()
    return _NC


def _prep_inputs(x, lora_A, lora_B):
    x2d = np.asarray(x, dtype=np.float32).reshape(TOK, D_IN)
    A = np.asarray(lora_A, dtype=np.float32)
    Bm = np.asarray(lora_B, dtype=np.float32)
    atp = np.ascontiguousarray(
        A.T.reshape(NC_D, 128, R).transpose(1, 0, 2).reshape(128, NC_D * R)
    ).astype(NP_BF16)
    btv = np.ascontiguousarray(Bm.T).astype(NP_BF16)
    xs_list = []
    for i in range(N_CORES):
        shard = x2d[i * TPC : (i + 1) * TPC].astype(NP_BF16)  # [2048, 4096]
        # [s, t, g, j, p] -> [s, g, p, j, t]
        v = shard.reshape(NSPLIT, TPS, NGS, CPS, 128).transpose(0, 2, 4, 3, 1)
        xs_list.append(
            np.ascontiguousarray(v).reshape(NSPLIT, NGS, 128, CPS * TPS)
        )
    return xs_list, atp, btv


def kernel(x, lora_A, lora_B, _trace=False, _trace_kwargs=None):
    nc = _get_nc()
    xs_list, atp, btv = _prep_inputs(x, lora_A, lora_B)
    in_maps = [
        {"xs": xs_list[i], "atp": atp, "bt": btv} for i in range(N_CORES)
    ]
    res = run_bass_kernel_spmd(
        nc, in_maps, list(range(N_CORES)), trace=_trace, **(_trace_kwargs or {})
    )
    y = np.concatenate(
        [np.asarray(res.results[i]["ys"]) for i in range(N_CORES)], axis=0
    )
    out = y.astype(np.float32).reshape(B, S, D_OUT)
    if _trace:
        return out, res
    return out
